# revision 1
# baseline (speedup 1.0000x reference)
"""Dev wrapper: kernel(**inputs) -> (output, action, final_h).

Final shipped version inlines work/gru_kernel.py + work/host_side.py.
"""
import sys

sys.path.insert(0, "/root/problem/work")
sys.path.insert(0, "/opt/trn_rl_repo")

import numpy as np

import gru_kernel
import host_side
from concourse.bass_utils import run_bass_kernel_spmd

SWEEPS = 8
C = 4
_cache = {}


def _get_nc():
    key = (SWEEPS, C)
    if key not in _cache:
        _cache[key] = gru_kernel.build(sweeps=SWEEPS, C=C)
    return _cache[key]


def _run(in_maps, trace=False):
    nc = _get_nc()
    return run_bass_kernel_spmd(nc, in_maps, list(range(8)), trace=trace)


def kernel(x, h0, Wi, Wh, bh, Wd, bd, seed):
    in_maps = host_side.make_in_maps(x, h0, Wi, Wh, bh, Wd, bd, C)
    res = _run(in_maps, trace=False)
    output, final_h = host_side.assemble(res.results, C)
    action = host_side.sample_action(output[:, :2], int(np.asarray(seed)))
    return output, action, final_h


def profile_exec_ns(x, h0, Wi, Wh, bh, Wd, bd, seed):
    """Run once more with NTFF tracing; returns exec_time_ns or None."""
    in_maps = host_side.make_in_maps(x, h0, Wi, Wh, bh, Wd, bd, C)
    try:
        res = _run(in_maps, trace=True)
        return res.exec_time_ns
    except Exception as e:  # noqa: BLE001
        print(f"profile failed: {e}")
        return None
